# revision 20
# baseline (speedup 1.0000x reference)
"""Trainium2 Bass kernel for BatchedFerroelectricBasis.

Math (restructured from the reference):
  switch_up cancels in `target`:
      target = su - sl + (1 - su - sl) = 1 - 2*sl
      bm     = ALPHA + (1-ALPHA)*target = 1 - 0.4*sl
      sl     = (1 - sigmoid(10*(x - prev))) * sigmoid(-10*x - 10*Ec)
             = g * cneg,   g = sigmoid(-10*(x - prev))
  basis = Ps*tanh(k*x + k*Ec - 0.4*k*Ec*g*cneg) + bias
  out[b,o] = sum_{i,n} coef*basis
           = sum_{i,n} P*tanh(k*(x - q) + k*Ec) + sum_{i,n} bias*coef
      with P = Ps*coef, q = 0.4*Ec*g*cneg

Layout: i (=128) on partitions, b (=512) on the free dim. The 512 (o,n)
pairs are sharded 8 ways (tensor parallel; 8 consecutive o per core).
Per (o,n): one ACT sigmoid (folds -10*Ec via per-partition bias), one
DVE scalar_tensor_tensor (q), one tensor-tensor subtract (v = x - q),
one ACT tanh (folds k / k*Ec via per-partition scale/bias), and one PE
matvec accumulating sum_i P*t into a PSUM row per o. The lag-1 `prev`
term only enters through g, computed once from x with a shifted AP.
"""

import numpy as np

B, I, O, NB = 512, 128, 64, 8
NCORES = 8
O_LOC = O // NCORES          # 8 output cols per core
ON_LOC = O_LOC * NB          # 64 (o,n) pairs per core

_CACHE: dict = {}


def _emit_body(nc, tc, mybir, dram, rep, abl=(), opts=None):
    """Emit one full kernel body (loads + compute + store).

    `abl` is a set of ablation flags used only for timing attribution
    experiments; the graded kernel always uses abl=().
    `opts`: {"biasmm": bool (ones-matmul per o vs folded scalar add),
             "gpsub": int (every Nth v-subtract goes to gpsimd; 0=never),
             "bufs": int work-pool buffers}
    """
    f32 = mybir.dt.float32
    Alu = mybir.AluOpType
    Act = mybir.ActivationFunctionType
    import concourse.tile as tile  # noqa: F401

    opts = dict(opts or {})
    biasmm = opts.get("biasmm", True)
    gpsub = opts.get("gpsub", 0)
    bufs = opts.get("bufs", 4)

    with (
        tc.tile_pool(name=f"persist{rep}", bufs=1) as persist,
        tc.tile_pool(name=f"work{rep}", bufs=bufs) as work,
        tc.tile_pool(name=f"ppool{rep}", bufs=1, space="PSUM") as ppool,
    ):
        xT = persist.tile([I, B], f32)
        nc.sync.dma_start(xT, dram["xT"])
        kp = persist.tile([I, ON_LOC], f32)
        nc.sync.dma_start(kp, dram["kk"])
        Ecp = persist.tile([I, ON_LOC], f32)
        nc.sync.dma_start(Ecp, dram["Ec"])
        Psp = persist.tile([I, ON_LOC], f32)
        nc.sync.dma_start(Psp, dram["Ps"])
        biasp = persist.tile([I, ON_LOC], f32)
        nc.sync.dma_start(biasp, dram["bias"])
        coefp = persist.tile([I, ON_LOC], f32)
        nc.sync.dma_start(coefp, dram["coef"])

        # g = sigmoid(-10*(x - prev)); prev[b] = x[b-1], prev[0] = 0
        d = persist.tile([I, B], f32)
        nc.scalar.copy(d[:, 0:1], xT[:, 0:1])
        nc.vector.tensor_sub(d[:, 1:B], xT[:, 1:B], xT[:, 0:B - 1])
        gT = persist.tile([I, B], f32)
        nc.scalar.activation(gT, d, Act.Sigmoid, bias=0.0, scale=-10.0)

        # derived per-(o,n) per-partition columns
        mEc10 = persist.tile([I, ON_LOC], f32)   # -10*Ec
        nc.vector.tensor_scalar_mul(mEc10, Ecp, -10.0)
        qc = persist.tile([I, ON_LOC], f32)      # 0.4*Ec
        nc.vector.tensor_scalar_mul(qc, Ecp, 0.4)
        kEc = persist.tile([I, ON_LOC], f32)     # k*Ec
        nc.vector.tensor_mul(kEc, kp, Ecp)
        Pw = persist.tile([I, ON_LOC], f32)      # Ps*coef
        nc.vector.tensor_mul(Pw, Psp, coefp)
        bcv = persist.tile([I, O_LOC, NB], f32)  # bias*coef
        nc.vector.tensor_mul(
            bcv, biasp[:].rearrange("p (o n) -> p o n", n=NB),
            coefp[:].rearrange("p (o n) -> p o n", n=NB))
        bcs = persist.tile([I, O_LOC], f32)      # sum_n bias*coef
        nc.vector.tensor_reduce(bcs, bcv, axis=mybir.AxisListType.X,
                                op=Alu.add)
        if biasmm:
            ones = persist.tile([I, B], f32)
            nc.vector.memset(ones, 1.0)
        else:
            # bct[0, o] = sum_i bcs[i, o] via one tiny matvec; the PSUM
            # scratch shares acc0's bank slot (used strictly before it).
            onescol = persist.tile([I, 1], f32)
            nc.vector.memset(onescol, 1.0)
            bct = persist.tile([1, O_LOC], f32)
            bct_ps = ppool.tile([128, O_LOC], f32, name=f"bct_ps{rep}",
                                tag="acc0")
            nc.tensor.matmul(bct_ps[0:1, :], lhsT=onescol, rhs=bcs,
                             start=True, stop=True)
            nc.vector.tensor_copy(bct, bct_ps[0:1, :])

        # one PSUM bank per output column o (PE writes must start at a
        # quadrant base partition, so row o of a shared bank is illegal)
        accs = [ppool.tile([128, B], f32, name=f"acc{rep}_{o}",
                           tag=f"acc{o}") for o in range(O_LOC)]

        F = 32 if "tiny" in abl else B
        for o in range(O_LOC):
            acc = accs[o]
            for n in range(NB):
                on = o * NB + n
                if "nosig" not in abl:
                    cneg = work.tile([I, B], f32)
                    nc.scalar.activation(cneg[:, 0:F], xT[:, 0:F],
                                         Act.Sigmoid,
                                         bias=mEc10[:, on:on + 1],
                                         scale=-10.0)
                else:
                    cneg = gT
                if "nostt" not in abl:
                    q = work.tile([I, B], f32)
                    nc.vector.scalar_tensor_tensor(
                        q[:, 0:F], cneg[:, 0:F], qc[:, on:on + 1],
                        gT[:, 0:F], op0=Alu.mult, op1=Alu.mult)
                else:
                    q = cneg
                if "nosub" not in abl:
                    v = work.tile([I, B], f32)
                    sub_eng = (nc.gpsimd if (gpsub and on % gpsub == 0)
                               else nc.vector)
                    sub_eng.tensor_sub(v[:, 0:F], xT[:, 0:F], q[:, 0:F])
                else:
                    v = q
                if "notanh" not in abl:
                    t = work.tile([I, B], f32)
                    nc.scalar.activation(t[:, 0:F], v[:, 0:F], Act.Tanh,
                                         bias=kEc[:, on:on + 1],
                                         scale=kp[:, on:on + 1])
                else:
                    t = v
                if "nomm" not in abl or n == 0:
                    nc.tensor.matmul(acc[0:1, 0:F], lhsT=Pw[:, on:on + 1],
                                     rhs=t[:, 0:F], start=(n == 0),
                                     stop=(not biasmm and n == NB - 1))
            if biasmm:
                nc.tensor.matmul(acc[0:1, 0:F], lhsT=bcs[:, o:o + 1],
                                 rhs=ones[:, 0:F], start=False, stop=True)

        outt = persist.tile([1, O_LOC * B], f32)
        for o in range(O_LOC):
            dst = outt[:, o * B:(o + 1) * B]
            if biasmm:
                if o % 2 == 0:
                    nc.scalar.copy(dst, accs[o][0:1, :])
                else:
                    nc.vector.tensor_copy(dst, accs[o][0:1, :])
            else:
                # copy + add the bias*coef column sum in one op
                if o % 2 == 0:
                    nc.scalar.activation(dst, accs[o][0:1, :], Act.Identity,
                                         bias=bct[0:1, o:o + 1], scale=1.0)
                else:
                    nc.vector.tensor_scalar_add(dst, accs[o][0:1, :],
                                                bct[0:1, o:o + 1])
        nc.sync.dma_start(dram["out"], outt)


def _build_module(reps=1, abl=(), opts=None):
    import concourse.bacc as bacc
    import concourse.tile as tile
    from concourse import mybir

    f32 = mybir.dt.float32
    nc = bacc.Bacc("TRN2", target_bir_lowering=False, debug=False,
                   num_devices=NCORES)

    dram = {
        "xT": nc.dram_tensor("xT", [I, B], f32, kind="ExternalInput").ap(),
        "kk": nc.dram_tensor("kk", [I, ON_LOC], f32,
                             kind="ExternalInput").ap(),
        "Ec": nc.dram_tensor("Ec", [I, ON_LOC], f32,
                             kind="ExternalInput").ap(),
        "Ps": nc.dram_tensor("Ps", [I, ON_LOC], f32,
                             kind="ExternalInput").ap(),
        "bias": nc.dram_tensor("bias", [I, ON_LOC], f32,
                               kind="ExternalInput").ap(),
        "coef": nc.dram_tensor("coef", [I, ON_LOC], f32,
                               kind="ExternalInput").ap(),
        "out": nc.dram_tensor("out", [1, O_LOC * B], f32,
                              kind="ExternalOutput").ap(),
    }

    with tile.TileContext(nc) as tc:
        for rep in range(reps):
            _emit_body(nc, tc, mybir, dram, rep, abl=abl, opts=opts)

    nc.compile()
    return nc


def _get_module():
    if "nc" not in _CACHE:
        _CACHE["nc"] = _build_module()
    return _CACHE["nc"]


def _make_in_maps(x, k, Ec, Ps, bias, coef):
    xT = np.ascontiguousarray(np.asarray(x, dtype=np.float32).T)  # [I, B]
    flat = {
        "kk": np.asarray(k, dtype=np.float32).reshape(I, O * NB),
        "Ec": np.asarray(Ec, dtype=np.float32).reshape(I, O * NB),
        "Ps": np.asarray(Ps, dtype=np.float32).reshape(I, O * NB),
        "bias": np.asarray(bias, dtype=np.float32).reshape(I, O * NB),
        "coef": np.asarray(coef, dtype=np.float32).reshape(I, O * NB),
    }
    in_maps = []
    for c in range(NCORES):
        sl = slice(c * ON_LOC, (c + 1) * ON_LOC)
        m = {"xT": xT}
        for name, arr in flat.items():
            m[name] = np.ascontiguousarray(arr[:, sl])
        in_maps.append(m)
    return in_maps


def _run(x, k, Ec, Ps, bias, coef, trace=False):
    from concourse.bass_utils import run_bass_kernel_spmd

    nc = _get_module()
    in_maps = _make_in_maps(x, k, Ec, Ps, bias, coef)
    res = run_bass_kernel_spmd(nc, in_maps, core_ids=list(range(NCORES)),
                               trace=trace)
    full = np.empty((B, O), dtype=np.float32)
    for c in range(NCORES):
        full[:, c * O_LOC:(c + 1) * O_LOC] = \
            res.results[c]["out"].reshape(O_LOC, B).T
    return full, res.exec_time_ns


def kernel(x, k, Ec, Ps, bias, coef):
    out, _ = _run(x, k, Ec, Ps, bias, coef)
    return out
